# revision 1
# baseline (speedup 1.0000x reference)
"""Trainium2 Bass kernel for nn_MultiHeadAttention (B=2, L=2048, D=1024, H=16).

Sharding: 8 cores = 2 batches x 4 head-groups (4 heads each, tensor parallel).
Host compacts masked-out key positions (mask==0 keys are removed, not masked),
pads to a 128 multiple. Per core:
  QT = (Wq_g @ xq^T + bq)/8            [256, 2048]    (fp32r, 2 partition tiles)
  KT = Wk_g @ xk_c^T + bk              [256, LK]      (fp32r)
  V  = xv_c @ Wv_g^T + bv (ones-aug)   [LK, 4x(64+1)] (bf16, per kpos-tile)
  S^T[kpos,q] = KT_h^T.T @ QT_h   (K=64, tile_position row-packed head pairs)
  P = exp(S^T + padmask)               (bf16; ACT reads scores from PSUM)
  O^T_h = V_h.T @ P  (M=65: ones row of V yields softmax denominators free)
  normalize: denom row -> gpsimd partition_broadcast -> DVE recip * O
  out_partial = O^T.T @ Wo_g^T         [2048, 1024]   (fp32r x fp32r -> fp32)
Host sums the 4 head-group partials per batch and adds bo.

Notes: all fp32r tensors carry plain fp32 bits (PE rounds internally).
The x streams and Wq/Wk/Wv are shipped bf16 (halves the input DMA that
gates the attention start; projections still accumulate in fp32 PSUM).
The ones column of V is written by DVE memset, NOT DMA - a strided DMA
there read-modify-writes SBUF lines concurrently with the DVE data
writes and corrupts first-execution results.
"""
import sys

sys.path.insert(0, "/opt/trn_rl_repo")

import numpy as np

B, L, D = 2, 2048, 1024
NH, DK = 16, 64
N_CORES = 8
GROUPS = 4          # head groups (cores per batch)
DQ = D // GROUPS    # 256 dims per group
HL = 4              # heads per group
T = L               # query tokens per core

_CACHE = {}
DEBUG_DUMP = False


def _build(LK):
    import concourse.bacc as bacc
    import concourse.mybir as mybir
    import concourse.tile as tile

    FR = mybir.dt.float32r
    F32 = mybir.dt.float32
    BF = mybir.dt.bfloat16
    AF = mybir.ActivationFunctionType

    NKT = LK // 128          # kpos tiles
    KB = [(i, min(1024, LK - i)) for i in range(0, LK, 1024)]  # key chunks
    QB = [(i, 1024) for i in range(0, T, 1024)]                # query chunks

    nc = bacc.Bacc("TRN2", target_bir_lowering=False, debug=False,
                   num_devices=N_CORES)

    xqT = nc.dram_tensor("xqT", [D, T], BF, kind="ExternalInput").ap()
    xkT = nc.dram_tensor("xkT", [D, LK], BF, kind="ExternalInput").ap()
    xvT = nc.dram_tensor("xvT", [D, LK], BF, kind="ExternalInput").ap()
    wq = nc.dram_tensor("wq", [D, DQ], BF, kind="ExternalInput").ap()
    wk = nc.dram_tensor("wk", [D, DQ], BF, kind="ExternalInput").ap()
    wv = nc.dram_tensor("wv", [D, DQ], BF, kind="ExternalInput").ap()
    wo = nc.dram_tensor("wo", [DQ, D], FR, kind="ExternalInput").ap()
    bqs = nc.dram_tensor("bqs", [128, 2], F32, kind="ExternalInput").ap()
    bks = nc.dram_tensor("bks", [128, 2], F32, kind="ExternalInput").ap()
    bvf = nc.dram_tensor("bvf", [128, DQ], F32, kind="ExternalInput").ap()
    padm = nc.dram_tensor("padm", [128, NKT], F32, kind="ExternalInput").ap()
    out = nc.dram_tensor("out", [T, D], F32, kind="ExternalOutput").ap()
    if DEBUG_DUMP:
        dqt = nc.dram_tensor("dqt", [128, 2, T], F32, kind="ExternalOutput").ap()
        dkt = nc.dram_tensor("dkt", [128, 2, LK], F32, kind="ExternalOutput").ap()
        dbgvt = nc.dram_tensor("dvt", [128, (LK // 128) * HL * 65], mybir.dt.bfloat16,
                             kind="ExternalOutput").ap()
        dot = nc.dram_tensor("dot", [128, 2, T], F32,
                             kind="ExternalOutput").ap()

    with tile.TileContext(nc) as tc:
        with tc.tile_pool(name="wsb", bufs=1) as wsb, \
             tc.tile_pool(name="per", bufs=1) as per, \
             tc.tile_pool(name="xs", bufs=3) as xsp, \
             tc.tile_pool(name="es", bufs=2) as esp, \
             tc.tile_pool(name="sm", bufs=2) as smp, \
             tc.tile_pool(name="pa", bufs=3, space="PSUM") as pap, \
             tc.tile_pool(name="pb", bufs=2, space="PSUM") as pbp:

            # ---- persistent loads
            twq = wsb.tile([128, 8, DQ], BF, tag="twq")
            twk = wsb.tile([128, 8, DQ], BF, tag="twk")
            twv = wsb.tile([128, 8, DQ], BF, tag="twv")
            two = wsb.tile([128, 2, D], FR, tag="two")
            tbq = wsb.tile([128, 2], F32, tag="tbq")
            tbk = wsb.tile([128, 2], F32, tag="tbk")
            tbvf = wsb.tile([128, DQ], F32, tag="tbvf")
            tpad = wsb.tile([128, NKT], F32, tag="tpad")
            nc.sync.dma_start(twq[:], wq.rearrange("(a p) m -> p a m", p=128))
            nc.sync.dma_start(twk[:], wk.rearrange("(a p) m -> p a m", p=128))
            nc.sync.dma_start(twv[:], wv.rearrange("(a p) m -> p a m", p=128))
            nc.sync.dma_start(two[:], wo.rearrange("(a p) n -> p a n", p=128))
            nc.sync.dma_start(tbq[:], bqs[:])
            nc.sync.dma_start(tbk[:], bks[:])
            nc.sync.dma_start(tbvf[:], bvf[:])
            nc.sync.dma_start(tpad[:], padm[:])

            # ---- persistent intermediates
            QT = [per.tile([128, T], FR, tag=f"QT{p}", name=f"QT{p}") for p in range(2)]
            KT = [per.tile([128, LK], FR, tag=f"KT{p}", name=f"KT{p}") for p in range(2)]
            Vt = per.tile([128, NKT, HL * 65], BF, tag="Vt")
            OT = [per.tile([128, T], FR, tag=f"OT{p}", name=f"OT{p}") for p in range(2)]

            xkT_r = xkT.rearrange("(a p) n -> p a n", p=128)
            xvT_r = xvT.rearrange("(a p) n -> p a n", p=128)
            xqT_r = xqT.rearrange("(a p) n -> p a n", p=128)

            def qproj(qb, qw):
                xq_t = xsp.tile([128, 8, qw], BF, tag="xs", name=f"xq{qb}")
                for kt in range(8):
                    nc.sync.dma_start(xq_t[:, kt, :], xqT_r[:, kt, qb:qb + qw])
                for p in range(2):
                    ps = pap.tile([128, 1024], F32, tag="pa", name=f"psq{qb}{p}")
                    for h0 in range(0, qw, 512):
                        for kt in range(8):
                            nc.tensor.matmul(
                                ps[:, h0:h0 + 512],
                                twq[:, kt, p * 128:(p + 1) * 128],
                                xq_t[:, kt, h0:h0 + 512],
                                start=(kt == 0), stop=(kt == 7))
                    nc.vector.tensor_scalar(
                        QT[p][:, qb:qb + qw], ps[:],
                        0.125, tbq[:, p:p + 1],
                        mybir.AluOpType.mult, mybir.AluOpType.add)

            # ---- K projection: KT[p] = twk[p].T @ xkT (+bk via DVE)
            for cb, cw in KB:
                xk_t = xsp.tile([128, 8, cw], BF, tag="xs")
                for kt in range(8):
                    nc.sync.dma_start(xk_t[:, kt, :], xkT_r[:, kt, cb:cb + cw])
                for p in range(2):
                    ps = pap.tile([128, 1024], F32, tag="pa")
                    for h0 in range(0, cw, 512):
                        hw = min(512, cw - h0)
                        for kt in range(8):
                            nc.tensor.matmul(
                                ps[:, h0:h0 + hw],
                                twk[:, kt, p * 128:(p + 1) * 128],
                                xk_t[:, kt, h0:h0 + hw],
                                start=(kt == 0), stop=(kt == 7))
                    nc.vector.tensor_scalar(
                        KT[p][:, cb:cb + cw], ps[:, 0:cw],
                        1.0, tbk[:, p:p + 1],
                        mybir.AluOpType.mult, mybir.AluOpType.add)

            # Q projection for the first block goes ahead of V so the
            # attention phase (gated on QT+KT) starts as early as possible
            qproj(*QB[0])

            # ---- V projection
            for cb, cw in KB:
                xv_t = xsp.tile([128, 8, cw], BF, tag="xs")
                for kt in range(8):
                    nc.sync.dma_start(xv_t[:, kt, :], xvT_r[:, kt, cb:cb + cw])
                for tl in range(cw // 128):
                    tt = cb // 128 + tl
                    psv = pap.tile([128, DQ], F32, tag="pa")
                    for kt in range(8):
                        nc.tensor.matmul(
                            psv[:], xv_t[:, kt, tl * 128:(tl + 1) * 128],
                            twv[:, kt, :], start=(kt == 0), stop=(kt == 7))
                    nc.vector.tensor_add(
                        Vt[:, tt, :].rearrange("p (h c) -> p h c", h=HL)[:, :, 0:64],
                        psv[:].rearrange("p (h c) -> p h c", h=HL),
                        tbvf[:].rearrange("p (h c) -> p h c", h=HL))
                # ones columns of V tiles (DVE memset: same engine as the
                # data writes, so ordering is enforced; a strided DMA here
                # races DVE writes within shared SBUF lines)
                for tl in range(cw // 128):
                    tt = cb // 128 + tl
                    nc.vector.memset(
                        Vt[:, tt, :].rearrange("p (h c) -> p h c", h=HL)[:, :, 64:65],
                        1.0)

            # ---- main loop over query blocks
            for qb, qw in QB:
                if qb != QB[0][0]:
                    qproj(qb, qw)

                for p in range(2):
                    # stage A: scores + exp, head pair row-packed
                    es = [[None] * NKT, [None] * NKT]
                    for kt in range(NKT):
                        pss = [pap.tile([128, 1024], F32, tag="pa", name=f"pss{i}")
                               for i in range(2)]
                        for h0 in range(0, qw, 512):
                            for hh in range(2):
                                r = hh * 64
                                nc.tensor.matmul(
                                    pss[hh][:, h0:h0 + 512],
                                    KT[p][r:r + 64, kt * 128:(kt + 1) * 128],
                                    QT[p][r:r + 64, qb + h0:qb + h0 + 512],
                                    start=True, stop=True,
                                    tile_position=(r, 0))
                        for hh in range(2):
                            e = esp.tile([128, 1024], BF, tag=f"es{hh}_{kt}")
                            es[hh][kt] = e
                            nc.scalar.activation(e[:], pss[hh][:], AF.Exp,
                                                 bias=tpad[:, kt:kt + 1])
                    # stage B per head, 512-wide halves
                    for hh in range(2):
                        hl = 2 * p + hh
                        r = hh * 64
                        for h0 in range(0, qw, 512):
                            pso = pbp.tile([65, 512], F32, tag="pb")
                            for kt in range(NKT):
                                nc.tensor.matmul(
                                    pso[:], Vt[:, kt, hl * 65:hl * 65 + 65],
                                    es[hh][kt][:, h0:h0 + 512],
                                    start=(kt == 0), stop=(kt == NKT - 1))
                            dn = smp.tile([1, 512], F32, tag="dn")
                            nc.vector.tensor_copy(dn[:], pso[64:65, :])
                            db = smp.tile([64, 512], F32, tag="db")
                            nc.gpsimd.partition_broadcast(db[:], dn[:])
                            rc = smp.tile([64, 512], F32, tag="rc")
                            nc.vector.reciprocal(rc[:], db[:])
                            nc.vector.tensor_mul(
                                OT[p][r:r + 64, qb + h0:qb + h0 + 512],
                                pso[0:64, :], rc[:])

                # output projection for this query block
                for tl in range(qw // 128):
                    tt = (qb + tl * 128)
                    pso = pap.tile([128, 1024], F32, tag="pa")
                    for nh in range(2):
                        for dvt in range(2):
                            nc.tensor.matmul(
                                pso[:, nh * 512:(nh + 1) * 512],
                                OT[dvt][:, tt:tt + 128],
                                two[:, dvt, nh * 512:(nh + 1) * 512],
                                start=(dvt == 0), stop=(dvt == 1))
                    ost = smp.tile([128, 1024], F32, tag="ost")
                    nc.scalar.copy(ost[:], pso[:])
                    nc.sync.dma_start(out[tt:tt + 128, :], ost[:])

            if DEBUG_DUMP:
                for p in range(2):
                    nc.sync.dma_start(dqt[:, p, :], QT[p][:].bitcast(F32))
                    nc.sync.dma_start(dkt[:, p, :], KT[p][:].bitcast(F32))
                    nc.sync.dma_start(dot[:, p, :], OT[p][:].bitcast(F32))
                nc.sync.dma_start(dbgvt[:], Vt[:].rearrange("p a b -> p (a b)"))

    nc.compile()
    return nc


def _exec(nc, in_maps):
    from concourse import bass2jax
    return bass2jax.run_bass_via_pjrt(nc, in_maps, n_cores=N_CORES)


def _prep(query, key, value, mask, Wq, bq, Wk, bk, Wv, bv, Wo, bo):
    """Host-side sharding. Returns (LK, in_maps, meta)."""
    f32 = np.float32
    q3 = np.asarray(query, f32).reshape(B, L, D)
    k3 = np.asarray(key, f32).reshape(B, L, D)
    v3 = np.asarray(value, f32).reshape(B, L, D)
    mask = np.asarray(mask)

    idxs = [np.nonzero(mask[b])[0] for b in range(B)]
    lens = [len(ix) for ix in idxs]
    LK = max(128, ((max(lens) + 127) // 128) * 128)

    xqT, xkT, xvT, padm = [], [], [], []
    for b in range(B):
        xqT.append(np.ascontiguousarray(q3[b].T))
        kk = np.zeros((LK, D), f32)
        vv = np.zeros((LK, D), f32)
        kk[:lens[b]] = k3[b][idxs[b]]
        vv[:lens[b]] = v3[b][idxs[b]]
        xkT.append(np.ascontiguousarray(kk.T))
        xvT.append(np.ascontiguousarray(vv.T))
        pm = np.zeros(LK, f32)
        pm[lens[b]:] = -30000.0
        padm.append(np.ascontiguousarray(pm.reshape(LK // 128, 128).T))

    Wq, bq = np.asarray(Wq, f32), np.asarray(bq, f32)
    Wk, bk = np.asarray(Wk, f32), np.asarray(bk, f32)
    Wv, bv = np.asarray(Wv, f32), np.asarray(bv, f32)
    Wo = np.asarray(Wo, f32)

    gm = {}
    for g in range(GROUPS):
        sl = slice(g * DQ, (g + 1) * DQ)
        gm[g] = dict(
            wq=np.ascontiguousarray(Wq[sl, :].T),
            wk=np.ascontiguousarray(Wk[sl, :].T),
            wv=np.ascontiguousarray(Wv[sl, :].T),
            wo=np.ascontiguousarray(Wo[:, sl].T),
            bqs=np.ascontiguousarray((bq[sl] / 8.0).reshape(2, 128).T),
            bks=np.ascontiguousarray(bk[sl].reshape(2, 128).T),
            bvf=np.ascontiguousarray(np.broadcast_to(bv[sl], (128, DQ))),
        )

    import ml_dtypes
    bf16 = np.dtype(ml_dtypes.bfloat16)

    in_maps = []
    for c in range(N_CORES):
        b, g = c // GROUPS, c % GROUPS
        m = gm[g]
        in_maps.append({
            "xqT": xqT[b].astype(bf16), "xkT": xkT[b].astype(bf16),
            "xvT": xvT[b].astype(bf16),
            "wq": m["wq"].astype(bf16), "wk": m["wk"].astype(bf16),
            "wv": m["wv"].astype(bf16),
            "wo": m["wo"],
            "bqs": m["bqs"], "bks": m["bks"], "bvf": m["bvf"],
            "padm": padm[b],
        })
    return LK, in_maps


def kernel(query, key, value, mask, Wq, bq, Wk, bk, Wv, bv, Wo, bo):
    LK, in_maps = _prep(query, key, value, mask, Wq, bq, Wk, bk, Wv, bv, Wo, bo)
    if LK not in _CACHE:
        _CACHE[LK] = _build(LK)
    nc = _CACHE[LK]
    results = _exec(nc, in_maps)
    bo = np.asarray(bo, np.float32)
    out = np.zeros((B, L, D), np.float32)
    for c in range(N_CORES):
        out[c // GROUPS] += results[c]["out"]
    out += bo[None, None, :]
    return out



# revision 20
# speedup vs baseline: 3.9638x; 3.9638x over previous
"""Trainium2 Bass kernel for nn_MultiHeadAttention (B=2, L=2048, D=1024, H=16).

Sharding: 8 cores = 2 batches x 4 head-groups (4 heads each, tensor parallel).
Host compacts masked-out key positions (mask==0 keys are removed, not masked),
pads to a 128 multiple. Per core:
  QT = (Wq_g @ xq^T + bq)/8            [256, 2048]    (fp32r, 2 partition tiles)
  KT = Wk_g @ xk_c^T + bk              [256, LK]      (fp32r)
  V  = xv_c @ Wv_g^T + bv (ones-aug)   [LK, 4x(64+1)] (bf16, per kpos-tile)
  S^T[kpos,q] = KT_h^T.T @ QT_h   (K=64, tile_position row-packed head pairs)
  P = exp(S^T + padmask)               (bf16; ACT reads scores from PSUM)
  O^T_h = V_h.T @ P  (M=65: ones row of V yields softmax denominators free)
  normalize: recip(denom row) -> gpsimd partition_broadcast -> DVE mul
  out_partial = O^T.T @ Wo_g^T         [2048, 1024]   (fp32r x fp32r -> fp32)
Host sums the 4 head-group partials per batch and adds bo.

The main loop is software-pipelined at (query-block, head-pair) "step"
granularity: the Activation engine (exp, ~18us/step) is the attention-phase
bottleneck, so the PE instruction stream interleaves the NEXT step's score
matmuls with the PREVIOUS step's attn@V matmuls plus deferred V/Q/out
projection work, keeping PE busy while ACT drains the score tiles. PSUM is
budgeted to exactly 8 banks: scores 2 tiles x 2 banks, attn@V 2 x 1,
projections 2 x 1. The exp ACT instructions stay 1024 elements wide (one
[128, 2x512] PSUM tile per (kpos-tile, q-half)) to amortize ACT init.

Notes: all fp32r tensors carry plain fp32 bits (PE rounds internally).
The x streams and Wq/Wk/Wv are shipped bf16 (halves the input DMA that
gates the attention start; projections still accumulate in fp32 PSUM).
The ones column of V is written by DVE memset, NOT DMA - a strided DMA
there read-modify-writes SBUF lines concurrently with the DVE data
writes and corrupts first-execution results.
"""
import sys

sys.path.insert(0, "/opt/trn_rl_repo")

import numpy as np

B, L, D = 2, 2048, 1024
NH, DK = 16, 64
N_CORES = 8
GROUPS = 4          # head groups (cores per batch)
DQ = D // GROUPS    # 256 dims per group
HL = 4              # heads per group
T = L               # query tokens per core

_CACHE = {}


def _build(LK):
    import concourse.bacc as bacc
    import concourse.mybir as mybir
    import concourse.tile as tile

    FR = mybir.dt.float32r
    F32 = mybir.dt.float32
    BF = mybir.dt.bfloat16
    AF = mybir.ActivationFunctionType

    NKT = LK // 128          # kpos tiles
    KB = [(i, min(1024, LK - i)) for i in range(0, LK, 1024)]  # key chunks
    QB = [(i, 1024) for i in range(0, T, 1024)]                # query chunks

    nc = bacc.Bacc("TRN2", target_bir_lowering=False, debug=False,
                   num_devices=N_CORES)

    xqT = nc.dram_tensor("xqT", [D, T], BF, kind="ExternalInput").ap()
    xkT = nc.dram_tensor("xkT", [D, LK], BF, kind="ExternalInput").ap()
    xvT = nc.dram_tensor("xvT", [D, LK], BF, kind="ExternalInput").ap()
    wq = nc.dram_tensor("wq", [D, DQ], BF, kind="ExternalInput").ap()
    wk = nc.dram_tensor("wk", [D, DQ], BF, kind="ExternalInput").ap()
    wv = nc.dram_tensor("wv", [D, DQ], BF, kind="ExternalInput").ap()
    wo = nc.dram_tensor("wo", [DQ, D], FR, kind="ExternalInput").ap()
    # aux packs bqs[0:2] bks[2:4] bvf[4:260] padm[260:260+NKT] (1 DMA:
    # each dma_start costs ~625ns of serialized HWDGE descriptor setup)
    aux = nc.dram_tensor("aux", [128, 260 + NKT], F32,
                         kind="ExternalInput").ap()
    out = nc.dram_tensor("out", [T, D], BF, kind="ExternalOutput").ap()

    with tile.TileContext(nc) as tc:
        with tc.tile_pool(name="wsb", bufs=1) as wsb, \
             tc.tile_pool(name="per", bufs=1) as per, \
             tc.tile_pool(name="xs", bufs=3) as xsp, \
             tc.tile_pool(name="es", bufs=2) as esp, \
             tc.tile_pool(name="sm", bufs=2) as smp, \
             tc.tile_pool(name="ps", bufs=2, space="PSUM") as psp, \
             tc.tile_pool(name="av", bufs=2, space="PSUM") as avp, \
             tc.tile_pool(name="pq", bufs=2, space="PSUM") as pqp:

            # ---- persistent loads (two is needed last; aux is one DMA)
            twq = wsb.tile([128, 8, DQ], BF, tag="twq")
            twk = wsb.tile([128, 8, DQ], BF, tag="twk")
            twv = wsb.tile([128, 8, DQ], BF, tag="twv")
            two = wsb.tile([128, 2, D], FR, tag="two")
            taux = wsb.tile([128, 260 + NKT], F32, tag="taux")
            nc.sync.dma_start(taux[:], aux[:])
            nc.sync.dma_start(twk[:], wk.rearrange("(a p) m -> p a m", p=128))

            # ---- persistent intermediates
            QT = [per.tile([128, T], FR, tag=f"QT{p}", name=f"QT{p}") for p in range(2)]
            KT = [per.tile([128, LK], FR, tag=f"KT{p}", name=f"KT{p}") for p in range(2)]
            Vt = per.tile([128, NKT, HL * 65], BF, tag="Vt")
            OT = [per.tile([128, T], FR, tag=f"OT{p}", name=f"OT{p}") for p in range(2)]

            xkT_r = xkT.rearrange("(a p) n -> p a n", p=128)
            xvT_r = xvT.rearrange("(a p) n -> p a n", p=128)
            xqT_r = xqT.rearrange("(a p) n -> p a n", p=128)

            # ---- K projection: KT[p] = twk[p].T @ xkT (+bk via DVE)
            # x DMAs are column-block-major so the first 512-col block (and
            # with it the first PE matmul) lands ~4x sooner than whole-chunk.
            for cb, cw in KB:
                xk_t = xsp.tile([128, 8, cw], BF, tag="xs", name=f"xk{cb}")
                for h0 in range(0, cw, 512):
                    hw = min(512, cw - h0)
                    nc.sync.dma_start(xk_t[:, :, h0:h0 + hw],
                                      xkT_r[:, :, cb + h0:cb + h0 + hw])
                for h0 in range(0, cw, 512):
                    hw = min(512, cw - h0)
                    for p in range(2):
                        ps = psp.tile([128, 512], F32, tag="ps",
                                      name=f"psk{cb}{p}{h0}")
                        for kt in range(8):
                            nc.tensor.matmul(
                                ps[:, 0:hw],
                                twk[:, kt, p * 128:(p + 1) * 128],
                                xk_t[:, kt, h0:h0 + hw],
                                start=(kt == 0), stop=(kt == 7))
                        nc.vector.tensor_scalar(
                            KT[p][:, cb + h0:cb + h0 + hw], ps[:, 0:hw],
                            1.0, taux[:, 2 + p:3 + p],
                            mybir.AluOpType.mult, mybir.AluOpType.add)

            # ---- Q projection emitter (pq pool, [128,512] grain)
            def qproj_part(qb, p, h0):
                ps = pqp.tile([128, 512], F32, tag="pq", name=f"psq{qb}{p}{h0}")
                xq_t = xq_tiles[qb]
                for kt in range(8):
                    nc.tensor.matmul(
                        ps[:],
                        twq[:, kt, p * 128:(p + 1) * 128],
                        xq_t[:, kt, h0:h0 + 512],
                        start=(kt == 0), stop=(kt == 7))
                nc.vector.tensor_scalar(
                    QT[p][:, qb + h0:qb + h0 + 512], ps[:],
                    0.125, taux[:, p:p + 1],
                    mybir.AluOpType.mult, mybir.AluOpType.add)

            # xq DMA for qb0, then Q projection for qb0 (both p) up front
            xq_tiles = {}

            def xq_dma(qb):
                xq_t = xsp.tile([128, 8, 1024], BF, tag="xs", name=f"xq{qb}")
                xq_tiles[qb] = xq_t
                for h0 in (0, 512):
                    nc.sync.dma_start(
                        xq_t[:, :, h0:h0 + 512],
                        xqT_r[:, :, qb + h0:qb + h0 + 512])

            nc.sync.dma_start(twq[:], wq.rearrange("(a p) m -> p a m", p=128))
            xq_dma(QB[0][0])
            for h0 in (0, 512):
                for p in range(2):
                    qproj_part(QB[0][0], p, h0)

            # ---- V projection work items (deferred into step 0 slots)
            nc.sync.dma_start(twv[:], wv.rearrange("(a p) m -> p a m", p=128))
            xv_tiles = {}
            for cb, cw in KB:
                xv_t = xsp.tile([128, 8, cw], BF, tag="xs", name=f"xv{cb}")
                xv_tiles[cb] = xv_t
                for h0 in range(0, cw, 512):
                    hw = min(512, cw - h0)
                    nc.sync.dma_start(xv_t[:, :, h0:h0 + hw],
                                      xvT_r[:, :, cb + h0:cb + h0 + hw])
            nc.sync.dma_start(two[:], wo.rearrange("(a p) n -> p a n", p=128))

            def vproj_item(cb, tl):
                def go():
                    xv_t = xv_tiles[cb]
                    tt = cb // 128 + tl
                    psv = pqp.tile([128, 512], F32, tag="pq", name=f"psv{tt}")
                    for kt in range(8):
                        nc.tensor.matmul(
                            psv[:, 0:DQ], xv_t[:, kt, tl * 128:(tl + 1) * 128],
                            twv[:, kt, :], start=(kt == 0), stop=(kt == 7))
                    nc.vector.tensor_add(
                        Vt[:, tt, :].rearrange("p (h c) -> p h c", h=HL)[:, :, 0:64],
                        psv[:, 0:DQ].rearrange("p (h c) -> p h c", h=HL),
                        taux[:, 4:260].rearrange("p (h c) -> p h c", h=HL))
                    # ones column (DVE memset: same engine as the data writes,
                    # so ordering is enforced; a strided DMA here races DVE)
                    nc.vector.memset(
                        Vt[:, tt, :].rearrange("p (h c) -> p h c", h=HL)[:, :, 64:65],
                        1.0)
                return go

            # ---- attention pipeline -----------------------------------
            STEPS = [(qb, p) for qb, _ in QB for p in range(2)]
            es_tiles = {}   # step -> list per kt of [128, 2, 1024] bf16

            def scores_emit(si, kt, h0):
                qb, p = STEPS[si]
                pss = psp.tile([128, 1024], F32, tag="ps",
                               name=f"pss{si}_{kt}_{h0}")
                for hh in range(2):
                    r = hh * 64
                    nc.tensor.matmul(
                        pss[:, hh * 512:(hh + 1) * 512],
                        KT[p][r:r + 64, kt * 128:(kt + 1) * 128],
                        QT[p][r:r + 64, qb + h0:qb + h0 + 512],
                        start=True, stop=True,
                        tile_position=(r, 0))
                e = es_tiles[si][kt]
                nc.scalar.activation(
                    e[:, :, h0 // 512, :],
                    pss[:].rearrange("p (h q) -> p h q", h=2),
                    AF.Exp, bias=taux[:, 260 + kt:261 + kt])

            def av_item(si, h0, kt, pso_pair):
                qb, p = STEPS[si]

                def go():
                    for hh in range(2):
                        hl = 2 * p + hh
                        if kt == 0:
                            pso_pair[hh] = avp.tile(
                                [65, 512], F32, tag="av",
                                name=f"av{si}_{h0}_{hh}")
                        nc.tensor.matmul(
                            pso_pair[hh][:], Vt[:, kt, hl * 65:hl * 65 + 65],
                            es_tiles[si][kt][:, hh, h0 // 512, :],
                            start=(kt == 0), stop=(kt == NKT - 1))
                return go

            def norm_item(si, h0, pso_pair):
                qb, p = STEPS[si]

                def go():
                    for hh in range(2):
                        r = hh * 64
                        rc = smp.tile([1, 512], F32, tag="rc")
                        nc.vector.reciprocal(rc[:], pso_pair[hh][64:65, :])
                        db = smp.tile([64, 512], F32, tag="db")
                        nc.gpsimd.partition_broadcast(db[:], rc[:])
                        nc.vector.tensor_mul(
                            OT[p][r:r + 64, qb + h0:qb + h0 + 512],
                            pso_pair[hh][0:64, :], db[:])
                return go

            def outproj_item(qb, tl, nh, eng):
                # one 512-wide half of an output row-tile; nh==1 also fires
                # the (single, fused) DMA for the full [128, 1024] row-tile
                tt = qb + tl * 128

                def go():
                    pso = pqp.tile([128, 512], F32, tag="pq",
                                   name=f"pso{tt}_{nh}")
                    for dvt in range(2):
                        nc.tensor.matmul(
                            pso[:],
                            OT[dvt][:, tt:tt + 128],
                            two[:, dvt, nh * 512:(nh + 1) * 512],
                            start=(dvt == 0), stop=(dvt == 1))
                    # PSUM->SBUF bf16 cast spread across whichever engines
                    # have slack (ACT only once its exp stream is done)
                    if nh == 0:
                        ost_tiles[tt] = smp.tile([128, 1024], BF, tag="ost",
                                                 name=f"ost{tt}")
                    ost = ost_tiles[tt]
                    dst = ost[:, nh * 512:(nh + 1) * 512]
                    # (Pool/gpsimd cannot access PSUM on TRN2)
                    if eng == 0:
                        nc.vector.tensor_copy(dst, pso[:])
                    else:
                        nc.scalar.copy(dst, pso[:])
                    if nh == 1:
                        nc.sync.dma_start(out[tt:tt + 128, :], ost[:])
                return go

            ost_tiles = {}

            # deferred PE work for each step's 18 (kt, h0) slots
            def av_items_for(si):
                items = []
                for h0 in (0, 512):
                    pso_pair = [None, None]
                    for kt in range(NKT):
                        items.append(av_item(si, h0, kt, pso_pair))
                    items.append(norm_item(si, h0, pso_pair))
                return items

            deferred = {0: [], 1: [], 2: [], 3: [], 4: []}
            # step 0: V projection (9 tiles)
            deferred[0] = [vproj_item(cb, tl) for cb, cw in KB
                           for tl in range(cw // 128)]
            # step 1: attn@V of step 0 + Q projection for qb1
            deferred[1] = av_items_for(0) + \
                [lambda p=p, h0=h0: qproj_part(QB[1][0], p, h0)
                 for p in range(2) for h0 in (0, 512)]
            # steps 2, 3: attn@V of steps 1, 2 + out projection of qb0
            # (DVE/Pool copies; ACT is busy with exp)
            deferred[2] = av_items_for(1) + \
                [outproj_item(QB[0][0], tl, nh, 0)
                 for tl in range(4) for nh in range(2)]
            deferred[3] = av_items_for(2) + \
                [outproj_item(QB[0][0], tl, nh, 0)
                 for tl in range(4, 8) for nh in range(2)]
            # tail: attn@V of step 3 + out projection of qb1 (3-way copies,
            # ACT included - its exp stream is over). The first half of the
            # out projection is zipped into the second attn@V sweep so the
            # copy/DMA drain overlaps PE work instead of trailing it.
            av3 = av_items_for(3)
            op3a = [outproj_item(QB[1][0], tl, nh, (tl * 2 + nh) % 2 * 2)
                    for tl in range(4) for nh in range(2)]
            op3b = [outproj_item(QB[1][0], tl, nh, (tl * 2 + nh) % 2 * 2)
                    for tl in range(4, 8) for nh in range(2)]
            tail = av3[:10]                      # h0=0 sweep + its norm
            for j, it in enumerate(av3[10:]):    # h0=512 sweep + norm
                tail.append(it)
                if j < len(op3a):
                    tail.append(op3a[j])
            deferred[4] = tail + op3a[len(av3) - 10:] + op3b

            for si, (qb, p) in enumerate(STEPS):
                es_tiles[si] = [esp.tile([128, 2, 2, 512], BF,
                                         tag=f"es{kt}", name=f"es{si}_{kt}")
                                for kt in range(NKT)]
                if si == 1:
                    xq_dma(QB[1][0])
                items = deferred[si]
                di = 0
                nslots = NKT * 2
                for slot, (kt, h0) in enumerate(
                        (kt, h0) for kt in range(NKT) for h0 in (0, 512)):
                    scores_emit(si, kt, h0)
                    # spread deferred items evenly across the slots
                    want = (slot + 1) * len(items) // nslots
                    while di < want:
                        items[di]()
                        di += 1
                while di < len(items):
                    items[di]()
                    di += 1
            for it in deferred[4]:
                it()

    nc.compile()
    return nc


def _exec(nc, in_maps):
    from concourse import bass2jax
    return bass2jax.run_bass_via_pjrt(nc, in_maps, n_cores=N_CORES)


def _prep(query, key, value, mask, Wq, bq, Wk, bk, Wv, bv, Wo, bo):
    """Host-side sharding. Returns (LK, in_maps)."""
    f32 = np.float32
    q3 = np.asarray(query, f32).reshape(B, L, D)
    k3 = np.asarray(key, f32).reshape(B, L, D)
    v3 = np.asarray(value, f32).reshape(B, L, D)
    mask = np.asarray(mask)

    idxs = [np.nonzero(mask[b])[0] for b in range(B)]
    lens = [len(ix) for ix in idxs]
    LK = max(128, ((max(lens) + 127) // 128) * 128)

    xqT, xkT, xvT, padm = [], [], [], []
    for b in range(B):
        xqT.append(np.ascontiguousarray(q3[b].T))
        kk = np.zeros((LK, D), f32)
        vv = np.zeros((LK, D), f32)
        kk[:lens[b]] = k3[b][idxs[b]]
        vv[:lens[b]] = v3[b][idxs[b]]
        xkT.append(np.ascontiguousarray(kk.T))
        xvT.append(np.ascontiguousarray(vv.T))
        pm = np.zeros(LK, f32)
        pm[lens[b]:] = -30000.0
        padm.append(np.ascontiguousarray(pm.reshape(LK // 128, 128).T))

    Wq, bq = np.asarray(Wq, f32), np.asarray(bq, f32)
    Wk, bk = np.asarray(Wk, f32), np.asarray(bk, f32)
    Wv, bv = np.asarray(Wv, f32), np.asarray(bv, f32)
    Wo = np.asarray(Wo, f32)

    NKT = LK // 128
    gm = {}
    for g in range(GROUPS):
        sl = slice(g * DQ, (g + 1) * DQ)
        auxes = []
        for b in range(B):
            a = np.zeros((128, 260 + NKT), f32)
            a[:, 0:2] = (bq[sl] / 8.0).reshape(2, 128).T
            a[:, 2:4] = bk[sl].reshape(2, 128).T
            a[:, 4:260] = np.broadcast_to(bv[sl], (128, DQ))
            a[:, 260:260 + NKT] = padm[b]
            auxes.append(a)
        gm[g] = dict(
            wq=np.ascontiguousarray(Wq[sl, :].T),
            wk=np.ascontiguousarray(Wk[sl, :].T),
            wv=np.ascontiguousarray(Wv[sl, :].T),
            wo=np.ascontiguousarray(Wo[:, sl].T),
            aux=auxes,
        )

    import ml_dtypes
    bf16 = np.dtype(ml_dtypes.bfloat16)

    in_maps = []
    for c in range(N_CORES):
        b, g = c // GROUPS, c % GROUPS
        m = gm[g]
        in_maps.append({
            "xqT": xqT[b].astype(bf16), "xkT": xkT[b].astype(bf16),
            "xvT": xvT[b].astype(bf16),
            "wq": m["wq"].astype(bf16), "wk": m["wk"].astype(bf16),
            "wv": m["wv"].astype(bf16),
            "wo": m["wo"],
            "aux": m["aux"][b],
        })
    return LK, in_maps


def kernel(query, key, value, mask, Wq, bq, Wk, bk, Wv, bv, Wo, bo):
    LK, in_maps = _prep(query, key, value, mask, Wq, bq, Wk, bk, Wv, bv, Wo, bo)
    if LK not in _CACHE:
        _CACHE[LK] = _build(LK)
    nc = _CACHE[LK]
    results = _exec(nc, in_maps)
    bo = np.asarray(bo, np.float32)
    out = np.zeros((B, L, D), np.float32)
    for c in range(N_CORES):
        out[c // GROUPS] += np.asarray(results[c]["out"], np.float32)
    out += bo[None, None, :]
    return out


# revision 31
# speedup vs baseline: 6.0264x; 1.5204x over previous
"""Trainium2 Bass kernel for nn_MultiHeadAttention (B=2, L=2048, D=1024, H=16).

Sharding: 8 cores = 2 batches x 4 head-groups (4 heads each, tensor parallel).
Host compacts masked-out key positions (mask==0 keys are removed, not masked),
pads to a 128 multiple. Per core:
  QT = (Wq_g @ xq^T + bq)/8            [256, 2048]    (fp32r, 2 partition tiles)
  KT = Wk_g @ xk_c^T + bk              [256, LK]      (fp32r)
  V  = xv_c @ Wv_g^T + bv (ones-aug)   [LK, 4x(64+1)] (bf16, per kpos-tile)
  S^T[kpos,q] = KT_h^T.T @ QT_h   (K=64, tile_position row-packed head pairs)
  P = exp(S^T + padmask)               (bf16; ACT reads scores from PSUM)
  O^T_h = V_h.T @ P  (M=65: ones row of V yields softmax denominators free)
  normalize: recip(denom row) -> gpsimd partition_broadcast -> DVE mul
  out_partial = O^T.T @ Wo_g^T         [2048, 1024]   (fp32r x fp32r -> fp32)
Host sums the 4 head-group partials per batch and adds bo.

The main loop is software-pipelined at (query-block, head-pair) "step"
granularity: the Activation engine (exp, ~18us/step) is the attention-phase
bottleneck, so the PE instruction stream interleaves the NEXT step's score
matmuls with the PREVIOUS step's attn@V matmuls plus deferred V/Q/out
projection work, keeping PE busy while ACT drains the score tiles. PSUM is
budgeted to exactly 8 banks: scores 2 tiles x 2 banks, attn@V 2 x 1,
projections 2 x 1. The exp ACT instructions stay 1024 elements wide (one
[128, 2x512] PSUM tile per (kpos-tile, q-half)) to amortize ACT init.

Notes: all fp32r tensors carry plain fp32 bits (PE rounds internally).
The x streams and Wq/Wk/Wv are shipped bf16 (halves the input DMA that
gates the attention start; projections still accumulate in fp32 PSUM).
The ones column of V is written by DVE memset, NOT DMA - a strided DMA
there read-modify-writes SBUF lines concurrently with the DVE data
writes and corrupts first-execution results.
"""
import sys

sys.path.insert(0, "/opt/trn_rl_repo")

import numpy as np

B, L, D = 2, 2048, 1024
NH, DK = 16, 64
N_CORES = 8
GROUPS = 4          # head groups (cores per batch)
DQ = D // GROUPS    # 256 dims per group
HL = 4              # heads per group
T = L               # query tokens per core

_CACHE = {}
USE_FR = True   # False: QT/KT/OT/Wo in bf16 (frees SBUF; tests HW fp32r rate)


def _build(LK):
    import concourse.bacc as bacc
    import concourse.mybir as mybir
    import concourse.tile as tile

    if use_fr is None:
        use_fr = USE_FR
    FR = mybir.dt.float32r if use_fr else mybir.dt.bfloat16
    F32 = mybir.dt.float32
    BF = mybir.dt.bfloat16
    AF = mybir.ActivationFunctionType

    NKT = LK // 128          # kpos tiles
    KB = [(i, min(1024, LK - i)) for i in range(0, LK, 1024)]  # key chunks
    QB = [(i, 1024) for i in range(0, T, 1024)]                # query chunks

    nc = bacc.Bacc("TRN2", target_bir_lowering=False, debug=False,
                   num_devices=N_CORES)

    xqT = nc.dram_tensor("xqT", [D, T], BF, kind="ExternalInput").ap()
    xkT = nc.dram_tensor("xkT", [D, LK], BF, kind="ExternalInput").ap()
    xvT = nc.dram_tensor("xvT", [D, LK], BF, kind="ExternalInput").ap()
    wq = nc.dram_tensor("wq", [D, DQ], BF, kind="ExternalInput").ap()
    wk = nc.dram_tensor("wk", [D, DQ], BF, kind="ExternalInput").ap()
    wv = nc.dram_tensor("wv", [D, DQ], BF, kind="ExternalInput").ap()
    wo = nc.dram_tensor("wo", [DQ, D], FR, kind="ExternalInput").ap()
    # aux packs bqs[0:2] bks[2:4] bvf[4:260] padm[260:260+NKT] (1 DMA:
    # each dma_start costs ~625ns of serialized HWDGE descriptor setup)
    aux = nc.dram_tensor("aux", [128, 260 + NKT], F32,
                         kind="ExternalInput").ap()
    out = nc.dram_tensor("out", [T, D], BF, kind="ExternalOutput").ap()

    with tile.TileContext(nc) as tc:
        with tc.tile_pool(name="wsb", bufs=1) as wsb, \
             tc.tile_pool(name="per", bufs=1) as per, \
             tc.tile_pool(name="xs", bufs=3) as xsp, \
             tc.tile_pool(name="es", bufs=2) as esp, \
             tc.tile_pool(name="sm", bufs=3) as smp, \
             tc.tile_pool(name="ps", bufs=2, space="PSUM") as psp, \
             tc.tile_pool(name="av", bufs=2, space="PSUM") as avp, \
             tc.tile_pool(name="pq", bufs=2, space="PSUM") as pqp:

            # ---- persistent loads (two is needed last; aux is one DMA)
            twq = wsb.tile([128, 8, DQ], BF, tag="twq")
            twk = wsb.tile([128, 8, DQ], BF, tag="twk")
            twv = wsb.tile([128, 8, DQ], BF, tag="twv")
            two = wsb.tile([128, 2, D], FR, tag="two")
            taux = wsb.tile([128, 260 + NKT], F32, tag="taux")
            nc.sync.dma_start(taux[:], aux[:])
            nc.sync.dma_start(twk[:], wk.rearrange("(a p) m -> p a m", p=128))

            # ---- persistent intermediates
            QT = [per.tile([128, T], FR, tag=f"QT{p}", name=f"QT{p}") for p in range(2)]
            KT = [per.tile([128, LK], FR, tag=f"KT{p}", name=f"KT{p}") for p in range(2)]
            Vt = per.tile([128, NKT, HL * 65], BF, tag="Vt")
            OT = [per.tile([128, T], FR, tag=f"OT{p}", name=f"OT{p}") for p in range(2)]

            xkT_r = xkT.rearrange("(a p) n -> p a n", p=128)
            xvT_r = xvT.rearrange("(a p) n -> p a n", p=128)
            xqT_r = xqT.rearrange("(a p) n -> p a n", p=128)

            # ---- K projection: KT[p] = twk[p].T @ xkT (+bk via DVE)
            # x DMAs are column-block-major so the first 512-col block (and
            # with it the first PE matmul) lands ~4x sooner than whole-chunk.
            for cb, cw in KB:
                xk_t = xsp.tile([128, 8, cw], BF, tag="xs", name=f"xk{cb}")
                for h0 in range(0, cw, 512):
                    hw = min(512, cw - h0)
                    nc.sync.dma_start(xk_t[:, :, h0:h0 + hw],
                                      xkT_r[:, :, cb + h0:cb + h0 + hw])
                for h0 in range(0, cw, 512):
                    hw = min(512, cw - h0)
                    for p in range(2):
                        ps = psp.tile([128, 512], F32, tag="ps",
                                      name=f"psk{cb}{p}{h0}")
                        for kt in range(8):
                            nc.tensor.matmul(
                                ps[:, 0:hw],
                                twk[:, kt, p * 128:(p + 1) * 128],
                                xk_t[:, kt, h0:h0 + hw],
                                start=(kt == 0), stop=(kt == 7))
                        nc.vector.tensor_scalar(
                            KT[p][:, cb + h0:cb + h0 + hw], ps[:, 0:hw],
                            1.0, taux[:, 2 + p:3 + p],
                            mybir.AluOpType.mult, mybir.AluOpType.add)

            # ---- Q projection emitter (pq pool, [128,512] grain)
            def qproj_part(qb, p, h0):
                ps = pqp.tile([128, 512], F32, tag="pq", name=f"psq{qb}{p}{h0}")
                xq_t = xq_tiles[qb]
                for kt in range(8):
                    nc.tensor.matmul(
                        ps[:],
                        twq[:, kt, p * 128:(p + 1) * 128],
                        xq_t[:, kt, h0:h0 + 512],
                        start=(kt == 0), stop=(kt == 7))
                nc.vector.tensor_scalar(
                    QT[p][:, qb + h0:qb + h0 + 512], ps[:],
                    0.125, taux[:, p:p + 1],
                    mybir.AluOpType.mult, mybir.AluOpType.add)

            # xq DMA for qb0, then Q projection for qb0 (both p) up front
            xq_tiles = {}

            def xq_dma(qb):
                xq_t = xsp.tile([128, 8, 1024], BF, tag="xs", name=f"xq{qb}")
                xq_tiles[qb] = xq_t
                for h0 in (0, 512):
                    nc.sync.dma_start(
                        xq_t[:, :, h0:h0 + 512],
                        xqT_r[:, :, qb + h0:qb + h0 + 512])

            nc.sync.dma_start(twq[:], wq.rearrange("(a p) m -> p a m", p=128))
            xq_dma(QB[0][0])
            for h0 in (0, 512):
                for p in range(2):
                    qproj_part(QB[0][0], p, h0)

            # ---- V projection work items (deferred into step 0 slots)
            nc.sync.dma_start(twv[:], wv.rearrange("(a p) m -> p a m", p=128))
            xv_tiles = {}
            for cb, cw in KB:
                xv_t = xsp.tile([128, 8, cw], BF, tag="xs", name=f"xv{cb}")
                xv_tiles[cb] = xv_t
                for h0 in range(0, cw, 512):
                    hw = min(512, cw - h0)
                    nc.sync.dma_start(xv_t[:, :, h0:h0 + hw],
                                      xvT_r[:, :, cb + h0:cb + h0 + hw])
            nc.sync.dma_start(two[:], wo.rearrange("(a p) n -> p a n", p=128))

            def vproj_item(cb, tl):
                def go():
                    xv_t = xv_tiles[cb]
                    tt = cb // 128 + tl
                    psv = pqp.tile([128, 512], F32, tag="pq", name=f"psv{tt}")
                    for kt in range(8):
                        nc.tensor.matmul(
                            psv[:, 0:DQ], xv_t[:, kt, tl * 128:(tl + 1) * 128],
                            twv[:, kt, :], start=(kt == 0), stop=(kt == 7))
                    nc.vector.tensor_add(
                        Vt[:, tt, :].rearrange("p (h c) -> p h c", h=HL)[:, :, 0:64],
                        psv[:, 0:DQ].rearrange("p (h c) -> p h c", h=HL),
                        taux[:, 4:260].rearrange("p (h c) -> p h c", h=HL))
                    # ones column (DVE memset: same engine as the data writes,
                    # so ordering is enforced; a strided DMA here races DVE)
                    nc.vector.memset(
                        Vt[:, tt, :].rearrange("p (h c) -> p h c", h=HL)[:, :, 64:65],
                        1.0)
                return go

            # ---- attention pipeline -----------------------------------
            STEPS = [(qb, p) for qb, _ in QB for p in range(2)]
            es_tiles = {}   # step -> list per kt of [128, 2, 1024] bf16

            def scores_emit(si, kt, h0):
                qb, p = STEPS[si]
                pss = psp.tile([128, 1024], F32, tag="ps",
                               name=f"pss{si}_{kt}_{h0}")
                for hh in range(2):
                    r = hh * 64
                    nc.tensor.matmul(
                        pss[:, hh * 512:(hh + 1) * 512],
                        KT[p][r:r + 64, kt * 128:(kt + 1) * 128],
                        QT[p][r:r + 64, qb + h0:qb + h0 + 512],
                        start=True, stop=True,
                        tile_position=(r, 0))
                e = es_tiles[si][kt]
                nc.scalar.activation(
                    e[:, :, h0 // 512, :],
                    pss[:].rearrange("p (h q) -> p h q", h=2),
                    AF.Exp, bias=taux[:, 260 + kt:261 + kt])

            def av_item(si, h0, kt, pso_pair):
                qb, p = STEPS[si]

                def go():
                    for hh in range(2):
                        hl = 2 * p + hh
                        if kt == 0:
                            pso_pair[hh] = avp.tile(
                                [65, 512], F32, tag="av",
                                name=f"av{si}_{h0}_{hh}")
                        nc.tensor.matmul(
                            pso_pair[hh][:], Vt[:, kt, hl * 65:hl * 65 + 65],
                            es_tiles[si][kt][:, hh, h0 // 512, :],
                            start=(kt == 0), stop=(kt == NKT - 1))
                return go

            def norm_item(si, h0, pso_pair):
                qb, p = STEPS[si]

                def go():
                    for hh in range(2):
                        r = hh * 64
                        # copy PSUM->SBUF first so the attn@V accumulator
                        # bank frees after ~1us instead of after the whole
                        # recip->broadcast->mul chain (the next h0 sweep's
                        # PE matmuls wait on that bank)
                        dcp = smp.tile([65, 512], F32, tag="dcp",
                                       name=f"dcp{si}_{h0}_{hh}")
                        nc.vector.tensor_copy(dcp[:], pso_pair[hh][:])
                        rc = smp.tile([1, 512], F32, tag="rc")
                        nc.vector.reciprocal(rc[:], dcp[64:65, :])
                        db = smp.tile([64, 512], F32, tag="db")
                        nc.gpsimd.partition_broadcast(db[:], rc[:])
                        nc.vector.tensor_mul(
                            OT[p][r:r + 64, qb + h0:qb + h0 + 512],
                            dcp[0:64, :], db[:])
                return go

            def outproj_item(qb, tl, nh, eng):
                # one 512-wide half of an output row-tile; nh==1 also fires
                # the (single, fused) DMA for the full [128, 1024] row-tile
                tt = qb + tl * 128

                def go():
                    pso = pqp.tile([128, 512], F32, tag="pq",
                                   name=f"pso{tt}_{nh}")
                    for dvt in range(2):
                        nc.tensor.matmul(
                            pso[:],
                            OT[dvt][:, tt:tt + 128],
                            two[:, dvt, nh * 512:(nh + 1) * 512],
                            start=(dvt == 0), stop=(dvt == 1))
                    # PSUM->SBUF bf16 cast spread across whichever engines
                    # have slack (ACT only once its exp stream is done)
                    if nh == 0:
                        ost_tiles[tt] = smp.tile([128, 1024], BF, tag="ost",
                                                 name=f"ost{tt}")
                    ost = ost_tiles[tt]
                    dst = ost[:, nh * 512:(nh + 1) * 512]
                    # (Pool/gpsimd cannot access PSUM on TRN2)
                    if eng == 0:
                        nc.vector.tensor_copy(dst, pso[:])
                    else:
                        nc.scalar.copy(dst, pso[:])
                    if nh == 1:
                        nc.sync.dma_start(out[tt:tt + 128, :], ost[:])
                return go

            ost_tiles = {}

            # deferred PE work for each step's 18 (kt, h0) slots
            def av_items_for(si):
                items = []
                for h0 in (0, 512):
                    pso_pair = [None, None]
                    for kt in range(NKT):
                        items.append(av_item(si, h0, kt, pso_pair))
                    items.append(norm_item(si, h0, pso_pair))
                return items

            deferred = {0: [], 1: [], 2: [], 3: [], 4: []}
            # step 0: V projection (9 tiles)
            deferred[0] = [vproj_item(cb, tl) for cb, cw in KB
                           for tl in range(cw // 128)]
            # step 1: attn@V of step 0 + Q projection for qb1
            deferred[1] = av_items_for(0) + \
                [lambda p=p, h0=h0: qproj_part(QB[1][0], p, h0)
                 for p in range(2) for h0 in (0, 512)]
            # steps 2, 3: attn@V of steps 1, 2 + out projection of qb0
            # (DVE/Pool copies; ACT is busy with exp)
            deferred[2] = av_items_for(1) + \
                [outproj_item(QB[0][0], tl, nh, 0)
                 for tl in range(4) for nh in range(2)]
            deferred[3] = av_items_for(2) + \
                [outproj_item(QB[0][0], tl, nh, 0)
                 for tl in range(4, 8) for nh in range(2)]
            # tail: attn@V of step 3 + out projection of qb1 (3-way copies,
            # ACT included - its exp stream is over). The first half of the
            # out projection is zipped into the second attn@V sweep so the
            # copy/DMA drain overlaps PE work instead of trailing it.
            av3 = av_items_for(3)
            op3a = [outproj_item(QB[1][0], tl, nh, (tl * 2 + nh) % 2 * 2)
                    for tl in range(4) for nh in range(2)]
            op3b = [outproj_item(QB[1][0], tl, nh, (tl * 2 + nh) % 2 * 2)
                    for tl in range(4, 8) for nh in range(2)]
            tail = av3[:10]                      # h0=0 sweep + its norm
            for j, it in enumerate(av3[10:]):    # h0=512 sweep + norm
                tail.append(it)
                if j < len(op3a):
                    tail.append(op3a[j])
            deferred[4] = tail + op3a[len(av3) - 10:] + op3b

            for si, (qb, p) in enumerate(STEPS):
                es_tiles[si] = [esp.tile([128, 2, 2, 512], BF,
                                         tag=f"es{kt}", name=f"es{si}_{kt}")
                                for kt in range(NKT)]
                if si == 1:
                    xq_dma(QB[1][0])
                items = deferred[si]
                di = 0
                nslots = NKT * 2
                for slot, (kt, h0) in enumerate(
                        (kt, h0) for kt in range(NKT) for h0 in (0, 512)):
                    scores_emit(si, kt, h0)
                    # spread deferred items evenly across the slots
                    want = (slot + 1) * len(items) // nslots
                    while di < want:
                        items[di]()
                        di += 1
                while di < len(items):
                    items[di]()
                    di += 1
            for it in deferred[4]:
                it()


            if loop:
                with tc.For_i(0, loop):
                    emit(0)
            else:
                for rep in range(reps):
                    emit(rep)

    nc.compile()
    return nc


def _exec(nc, in_maps):
    from concourse import bass2jax
    return bass2jax.run_bass_via_pjrt(nc, in_maps, n_cores=N_CORES)


def _prep(query, key, value, mask, Wq, bq, Wk, bk, Wv, bv, Wo, bo):
    """Host-side sharding. Returns (LK, in_maps)."""
    f32 = np.float32
    q3 = np.asarray(query, f32).reshape(B, L, D)
    k3 = np.asarray(key, f32).reshape(B, L, D)
    v3 = np.asarray(value, f32).reshape(B, L, D)
    mask = np.asarray(mask)

    idxs = [np.nonzero(mask[b])[0] for b in range(B)]
    lens = [len(ix) for ix in idxs]
    LK = max(128, ((max(lens) + 127) // 128) * 128)

    xqT, xkT, xvT, padm = [], [], [], []
    for b in range(B):
        xqT.append(np.ascontiguousarray(q3[b].T))
        kk = np.zeros((LK, D), f32)
        vv = np.zeros((LK, D), f32)
        kk[:lens[b]] = k3[b][idxs[b]]
        vv[:lens[b]] = v3[b][idxs[b]]
        xkT.append(np.ascontiguousarray(kk.T))
        xvT.append(np.ascontiguousarray(vv.T))
        pm = np.zeros(LK, f32)
        pm[lens[b]:] = -30000.0
        padm.append(np.ascontiguousarray(pm.reshape(LK // 128, 128).T))

    Wq, bq = np.asarray(Wq, f32), np.asarray(bq, f32)
    Wk, bk = np.asarray(Wk, f32), np.asarray(bk, f32)
    Wv, bv = np.asarray(Wv, f32), np.asarray(bv, f32)
    Wo = np.asarray(Wo, f32)

    NKT = LK // 128
    gm = {}
    for g in range(GROUPS):
        sl = slice(g * DQ, (g + 1) * DQ)
        auxes = []
        for b in range(B):
            a = np.zeros((128, 260 + NKT), f32)
            a[:, 0:2] = (bq[sl] / 8.0).reshape(2, 128).T
            a[:, 2:4] = bk[sl].reshape(2, 128).T
            a[:, 4:260] = np.broadcast_to(bv[sl], (128, DQ))
            a[:, 260:260 + NKT] = padm[b]
            auxes.append(a)
        gm[g] = dict(
            wq=np.ascontiguousarray(Wq[sl, :].T),
            wk=np.ascontiguousarray(Wk[sl, :].T),
            wv=np.ascontiguousarray(Wv[sl, :].T),
            wo=np.ascontiguousarray(Wo[:, sl].T),
            aux=auxes,
        )

    import ml_dtypes
    bf16 = np.dtype(ml_dtypes.bfloat16)

    in_maps = []
    for c in range(N_CORES):
        b, g = c // GROUPS, c % GROUPS
        m = gm[g]
        in_maps.append({
            "xqT": xqT[b].astype(bf16), "xkT": xkT[b].astype(bf16),
            "xvT": xvT[b].astype(bf16),
            "wq": m["wq"].astype(bf16), "wk": m["wk"].astype(bf16),
            "wv": m["wv"].astype(bf16),
            "wo": m["wo"] if USE_FR else m["wo"].astype(bf16),
            "aux": m["aux"][b],
        })
    return LK, in_maps


def kernel(query, key, value, mask, Wq, bq, Wk, bk, Wv, bv, Wo, bo):
    LK, in_maps = _prep(query, key, value, mask, Wq, bq, Wk, bk, Wv, bv, Wo, bo)
    if LK not in _CACHE:
        _CACHE[LK] = _build(LK)
    nc = _CACHE[LK]
    results = _exec(nc, in_maps)
    bo = np.asarray(bo, np.float32)
    out = np.zeros((B, L, D), np.float32)
    for c in range(N_CORES):
        out[c // GROUPS] += np.asarray(results[c]["out"], np.float32)
    out += bo[None, None, :]
    return out
